# revision 1
# baseline (speedup 1.0000x reference)
"""Causal single-head attention (nn_AttentionHead) on 8 TRN2 NeuronCores.

Self-contained: kernel(**inputs) takes the full fp32 inputs and returns the
full [4, 4096, 64] output, distributing work across 8 cores internally.

Sharding: 8 cores = 4 batches x 2 key-parity shards. Core (b, h) computes,
for ALL 4096 queries of batch b, unnormalized flash-attention partials
(numerator [64] + denominator) over the keys in 128-row blocks of parity h;
the host sums the two partials per batch and normalizes (softmax without
max-subtraction is mathematically exact; scores here are O(1) so exp is
safe in fp32).

One SPMD program for all 8 cores. Per-core differences are data only:
  - xq: x[b]^T with columns permuted so the core's key blocks come first
  - smask: step masks for the ragged (parity-dependent) coverage of
    second-half query chunks
All matmuls run in float32r (~1e-4 rel err, full PE speed). The score
scale 1/8 and bias bq are folded into Q; bk is applied to K; bv is added
during the host combine (softmax-weight-invariant).
"""

import os
import sys
from contextlib import ExitStack

import numpy as np

for _p in ("/root/.axon_site/_ro/trn_rl_repo", "/opt/trn_rl_repo"):
    if os.path.isdir(_p) and _p not in sys.path:
        sys.path.append(_p)

import concourse.bacc as bacc
import concourse.tile as tile
from concourse import mybir

F32 = mybir.dt.float32
F32R = mybir.dt.float32r

B, T, C, H = 4, 4096, 1024, 64
KT = C // 128  # contraction tiles
NKEY = T // 2  # keys per core
NKT = NKEY // 128  # key tiles per core
NQC = T // 512  # query chunks
NB = T // 128  # 128-row blocks


# ---------------------------------------------------------------- device ----
def build(reps: int = 1):
    nc = bacc.Bacc("TRN2", target_bir_lowering=False, debug=False)

    xq = nc.dram_tensor("xq", [C, T], F32R, kind="ExternalInput")
    wq = nc.dram_tensor("wq", [C, H], F32R, kind="ExternalInput")
    wkv = nc.dram_tensor("wkv", [C, 128], F32R, kind="ExternalInput")  # [Wk|Wv]
    bq8 = nc.dram_tensor("bq8", [H, 1], F32, kind="ExternalInput")  # bq/8
    bkv = nc.dram_tensor("bkv", [128, 1], F32, kind="ExternalInput")  # [bk;0]
    ident = nc.dram_tensor("ident", [128, 128], F32R, kind="ExternalInput")
    smask = nc.dram_tensor("smask", [4, 128, 512], F32, kind="ExternalInput")

    outp = nc.dram_tensor("outp", [H + 1, T], F32, kind="ExternalOutput")

    with tile.TileContext(nc) as tc, ExitStack() as ctx:
        const = ctx.enter_context(tc.tile_pool(name="const", bufs=1))
        big = ctx.enter_context(tc.tile_pool(name="big", bufs=1))
        work = ctx.enter_context(tc.tile_pool(name="work", bufs=3))
        ps = ctx.enter_context(tc.tile_pool(name="ps", bufs=2, space="PSUM"))

        wqt = const.tile([128, KT, H], F32R)
        nc.sync.dma_start(out=wqt, in_=wq.ap().rearrange("(k p) m -> p k m", p=128))
        wkvt = const.tile([128, KT, 128], F32R)
        nc.sync.dma_start(out=wkvt, in_=wkv.ap().rearrange("(k p) m -> p k m", p=128))
        bq8t = const.tile([H, 1], F32)
        nc.sync.dma_start(out=bq8t, in_=bq8.ap())
        bkvt = const.tile([128, 1], F32)
        nc.sync.dma_start(out=bkvt, in_=bkv.ap())
        idt = const.tile([128, 128], F32R)
        nc.sync.dma_start(out=idt, in_=ident.ap())
        smt = const.tile([128, 4, 512], F32)
        nc.sync.dma_start(out=smt, in_=smask.ap().rearrange("m p t -> p m t"))
        onest = const.tile([128, 1], F32)
        nc.vector.memset(onest, 1.0)

        for _ in range(reps):
            _body(nc, big, work, ps, xq, outp, wqt, wkvt, bq8t, bkvt, idt, smt, onest)

    nc.compile()
    return nc


def _body(nc, big, work, ps, xq, outp, wqt, wkvt, bq8t, bkvt, idt, smt, onest):
    # xq resident, loaded in 4 chunks so compute can start early
    xqt = big.tile([128, KT, T], F32R, tag="xqt")
    xq_r = xq.ap().rearrange("(k p) t -> p k t", p=128)
    for i in range(4):
        sl = slice(1024 * i, 1024 * (i + 1))
        nc.sync.dma_start(out=xqt[:, :, sl], in_=xq_r[:, :, sl])

    kvt = big.tile([128, NKEY], F32R, tag="kvt")  # rows 0:64 K^T, 64:128 V^T
    qt = big.tile([H, T], F32R, tag="qt")
    vp = big.tile([128, NKT, H + 1], F32R, tag="vp")

    # kv-pass: [Wk|Wv] over the key columns (0:NKEY)
    for kc in range(NKEY // 512):
        sl = slice(512 * kc, 512 * (kc + 1))
        pkv = ps.tile([128, 512], F32, tag="proj")
        for k in range(KT):
            nc.tensor.matmul(
                pkv, lhsT=wkvt[:, k, :], rhs=xqt[:, k, sl],
                start=(k == 0), stop=(k == KT - 1),
            )
        nc.vector.tensor_scalar_add(kvt[:, sl], pkv, bkvt)

    # q-pass: Q^T = 0.125*(x Wq) + bq/8 over all columns
    for qc in range(NQC):
        sl = slice(512 * qc, 512 * (qc + 1))
        pq = ps.tile([H, 512], F32, tag="proj")
        for k in range(KT):
            nc.tensor.matmul(
                pq, lhsT=wqt[:, k, :], rhs=xqt[:, k, sl],
                start=(k == 0), stop=(k == KT - 1),
            )
        nc.vector.tensor_scalar(
            qt[:, sl], pq, 0.125, bq8t,
            op0=mybir.AluOpType.mult, op1=mybir.AluOpType.add,
        )

    # V natural ([V | 1] per key tile) via PE transpose
    for t in range(NKT):
        ptr = ps.tile([128, H], F32R, tag="proj")
        nc.tensor.transpose(
            ptr, kvt[64:128, 128 * t : 128 * (t + 1)], idt[64:128, 64:128]
        )
        nc.vector.tensor_copy(vp[:, t, 0:H], ptr)
        nc.vector.tensor_copy(vp[:, t, H : H + 1], onest)

    # attention: per 512-query chunk, stream k-tiles
    for qc in range(NQC):
        qsl = slice(512 * qc, 512 * (qc + 1))
        first_half = qc < NQC // 2
        cc = qc if first_half else qc - NQC // 2
        n_k = 4 * cc + 4
        acc = ps.tile([H + 1, 512], F32, tag="acc")
        for jp in range(n_k // 2):
            sp = ps.tile([128, 1024], F32, tag="sp")
            for u in range(2):
                j = 2 * jp + u
                nc.tensor.matmul(
                    sp[:, 512 * u : 512 * (u + 1)],
                    lhsT=kvt[0:64, 128 * j : 128 * (j + 1)],
                    rhs=qt[:, qsl],
                    start=True, stop=True,
                )
            pt = work.tile([128, 1024], F32R, tag="pt")
            nc.scalar.activation(pt, sp, mybir.ActivationFunctionType.Exp)
            for u in range(2):
                j = 2 * jp + u
                psl = slice(512 * u, 512 * (u + 1))
                d = j - (n_k - 4)
                if d >= 0:
                    if first_half:
                        nc.gpsimd.affine_select(
                            out=pt[:, psl], in_=pt[:, psl],
                            pattern=[[1, 512]], channel_multiplier=-1,
                            base=-128 * d, compare_op=mybir.AluOpType.is_ge,
                            fill=0.0,
                        )
                    else:
                        nc.vector.tensor_mul(pt[:, psl], pt[:, psl], smt[:, d, :])
                nc.tensor.matmul(
                    acc, lhsT=vp[:, j, :], rhs=pt[:, psl],
                    start=(j == 0), stop=(j == n_k - 1),
                )
        so = work.tile([H + 1, 512], F32, tag="so")
        nc.scalar.copy(so, acc)
        nc.sync.dma_start(out=outp.ap()[:, qsl], in_=so)


# ------------------------------------------------------------------ host ----
def _perm_cols(h):
    blocks = list(range(h, NB, 2)) + list(range(1 - h, NB, 2))
    return np.concatenate([np.arange(128 * g, 128 * (g + 1)) for g in blocks])


def _step_masks(h):
    m = np.zeros((4, 128, 512), dtype=np.float32)
    for t in range(4):
        for i in range(4):
            if (i >= t) if h == 0 else (i >= t + 1):
                m[t, :, 128 * i : 128 * (i + 1)] = 1.0
    return m


def _make_in_maps(batch_x, Wk, bk, Wq, bq, Wv):
    xT = np.ascontiguousarray(np.transpose(batch_x, (0, 2, 1)))
    wkv = np.ascontiguousarray(
        np.concatenate([Wk, Wv], axis=1).astype(np.float32)
    )
    wq_c = np.ascontiguousarray(Wq.astype(np.float32))
    bq8 = (bq.astype(np.float32) * 0.125).reshape(H, 1)
    bkv = np.concatenate(
        [bk.astype(np.float32), np.zeros(64, np.float32)]
    ).reshape(128, 1)
    ident = np.eye(128, dtype=np.float32)
    cols = {h: _perm_cols(h) for h in (0, 1)}
    masks = {h: _step_masks(h) for h in (0, 1)}
    return [
        {
            "xq": np.ascontiguousarray(xT[b][:, cols[h]]),
            "wq": wq_c,
            "wkv": wkv,
            "bq8": bq8,
            "bkv": bkv,
            "ident": ident,
            "smask": masks[h],
        }
        for b in range(B)
        for h in (0, 1)
    ]


def _combine(outps, bv):
    inv = {}
    for h in (0, 1):
        c = _perm_cols(h)
        inv[h] = np.empty_like(c)
        inv[h][c] = np.arange(T)
    out = np.empty((B, T, H), dtype=np.float32)
    for b in range(B):
        tot = np.zeros((H + 1, T), dtype=np.float64)
        for h in (0, 1):
            o = np.asarray(outps[2 * b + h], dtype=np.float64)
            tot += o[:, inv[h]]
        out[b] = (tot[0:H] / tot[H]).T + bv.astype(np.float64)
    return out


_CACHE = {}


def _get_nc():
    if "nc" not in _CACHE:
        _CACHE["nc"] = build(reps=1)
    return _CACHE["nc"]


def kernel(batch_x, Wk, bk, Wq, bq, Wv, bv):
    from concourse.bass_utils import run_bass_kernel_spmd

    batch_x = np.asarray(batch_x, dtype=np.float32)
    in_maps = _make_in_maps(
        batch_x, np.asarray(Wk), np.asarray(bk), np.asarray(Wq),
        np.asarray(bq), np.asarray(Wv),
    )
    nc = _get_nc()
    res = run_bass_kernel_spmd(nc, in_maps, core_ids=list(range(8)))
    outps = [res.results[c]["outp"] for c in range(8)]
    return _combine(outps, np.asarray(bv))
